# revision 12
# baseline (speedup 1.0000x reference)
"""MoE GemmaMLP (top-2 of 8 experts + shared expert) on 8 trn2 NeuronCores.

v4: bf16 matmuls, unified expert machinery, contiguous DMA layouts,
hg-interleaved down projection (consecutive matmuls share the stationary
a^T tile; PSUM split 5 gate/up + 3 down-scratch).

Sharding: expert-parallel with load balancing.  The host computes top-2
routing, chunks each expert's routed batches into pairs, and packs pairs
into weight-stream groups of <=2 pairs sharing one expert.  Each core runs:
two 2-pair expert groups (full I), ONE 2-pair shared-expert group over its
4 data-parallel batches (weights streamed like an expert's), and the
leftover single pairs as tensor-parallel slots (I/8 slice on every core).
The host sums tp partials and applies routing weights.

All matmul operands are bf16 (fp32 PSUM accumulation): full PE rate and
half the DMA bytes of fp32.  The host pre-tiles x and gate/up weights so
every DMA is contiguous >=2KB per partition:
  xt pairs:  [P, HT*2S]  (pair interleaved per h-tile; one DMA per pair)
  gate/up:   [P, NI*HT*128]  (i-tile-major; one 2KB/partition DMA per i-tile)
  down:      [I, H] row-sliced (already 2KB/partition contiguous)

Layout trick: x transposed to xT so every matmul is transpose-free:
  gate/up:  g^T[i,s] = sum_h Wg[h,i] * xT[h,s]   (lhsT = Wg tile, rhs = xT)
  down:     out[s,h] = sum_i a^T[i,s] * Wd[i,h]  (lhsT = a^T tile, rhs = Wd)
Pairs give 2*S = 512 moving columns (PSUM-bank max for fp32 output).
DMAs alternate between the SP and ACT HWDGE rings.
"""

import os
import numpy as np
import ml_dtypes
from contextlib import ExitStack

import concourse.bass as bass
import concourse.mybir as mybir
import concourse.tile as tile
from concourse import bacc
from concourse.bass_utils import run_bass_kernel_spmd

B, S, H, I, E = 32, 256, 1024, 4096, 8
TOP_K = 2
NUM_MOE_LAYERS = 12
NCORES = 8
HT = H // 128             # h-tiles
P = 128
S2 = 2 * S
NIF = I // P              # full-I i-tiles (32)
TPI = I // NCORES         # tp-slot i-columns per core

F32 = mybir.dt.float32
MM_DT = mybir.dt.bfloat16
NP_MM = ml_dtypes.bfloat16
GELU = mybir.ActivationFunctionType.Gelu_apprx_tanh

CHUNK = 8                 # i-tiles per down-accumulation chunk
GROUPS = ((2, 32), (2, 32), (2, 32), (1, 4), (1, 4))  # (pairs, i-tiles)


def _expert_group(nc, pools, xt_rows, wg_d, wu_d, wd_d, out_rows, n_pairs,
                  ni=NIF):
    """One weight-stream group: n_pairs pairs sharing one expert's weights.

    xt_rows: n_pairs DRAM APs [P, HT*S2] (pre-interleaved pair tiles)
    wg_d/wu_d: [P, ni*HT*128] i-tile-major lhsT layouts
    wd_d: [ni*128, H]
    out_rows: 2*n_pairs DRAM APs [S, H]
    """
    xt_p, psgu, pssc, tmp_p, at_p, ob_p = (
        pools[k] for k in ("xt", "psgu", "pssc", "tmp", "aT", "outsb"))
    NI = ni

    xt_t = []
    for pr in range(n_pairs):
        t = xt_p.tile([P, HT * S2], MM_DT, tag="xt", name=f"xt{pr}")
        eng = nc.sync if pr % 2 == 0 else nc.scalar
        eng.dma_start(t[:], xt_rows[pr])
        xt_t.append(t)

    # per-pair output accumulators [128 s, (ss, hg) * 512]
    out_sb = [ob_p.tile([P, 4 * 2 * 512], F32, tag="outsb",
                        name=f"osb{pr}") for pr in range(n_pairs)]

    for c0 in range(0, NI, CHUNK):
        chunk = range(c0, min(c0 + CHUNK, NI))
        ch_n = len(chunk)
        at_t = at_p.tile([P, CHUNK * n_pairs * S2], MM_DT, tag="aT")
        wd_ts = []
        for ci, i in enumerate(chunk):
            wg_t = pools["wg"].tile([P, HT * P], MM_DT, tag="wg")
            nc.sync.dma_start(wg_t[:],
                              wg_d[:, i * HT * P:(i + 1) * HT * P])
            wu_t = pools["wu"].tile([P, HT * P], MM_DT, tag="wu")
            nc.scalar.dma_start(wu_t[:],
                                wu_d[:, i * HT * P:(i + 1) * HT * P])
            ps_g = [psgu.tile([P, S2], F32, tag="ps", name=f"psg{pr}")
                    for pr in range(n_pairs)]
            ps_u = [psgu.tile([P, S2], F32, tag="ps", name=f"psu{pr}")
                    for pr in range(n_pairs)]
            for t in range(HT):
                for pr in range(n_pairs):
                    nc.tensor.matmul(ps_g[pr][:], wg_t[:, t * P:(t + 1) * P],
                                     xt_t[pr][:, t * S2:(t + 1) * S2],
                                     start=(t == 0), stop=(t == HT - 1))
            for t in range(HT):
                for pr in range(n_pairs):
                    nc.tensor.matmul(ps_u[pr][:], wu_t[:, t * P:(t + 1) * P],
                                     xt_t[pr][:, t * S2:(t + 1) * S2],
                                     start=(t == 0), stop=(t == HT - 1))
            wd_t = pools["wd"].tile([P, H], MM_DT, tag="wd")
            eng = nc.sync if i % 2 == 0 else nc.scalar
            eng.dma_start(wd_t[:], wd_d[i * P:(i + 1) * P, :])
            wd_ts.append(wd_t)
            for pr in range(n_pairs):
                tmp_g = tmp_p.tile([P, S2], F32, tag="tmp")
                nc.scalar.activation(tmp_g[:], ps_g[pr][:], GELU)
                col = (ci * n_pairs + pr) * S2
                nc.vector.tensor_mul(at_t[:, col:col + S2], tmp_g[:],
                                     ps_u[pr][:])

        # down for this chunk: accumulate into out_sb (hg-interleaved so
        # consecutive matmuls share the same stationary a^T tile)
        for pr in range(n_pairs):
            for ss in range(4):
                sc = [pssc.tile([P, 512], F32, tag="sc", name=f"sc{hg}")
                      for hg in range(2)]
                for ci in range(ch_n):
                    col = (ci * n_pairs + pr) * S2 + ss * P
                    for hg in range(2):
                        nc.tensor.matmul(sc[hg][:], at_t[:, col:col + P],
                                         wd_ts[ci][:, hg * 512:(hg + 1) * 512],
                                         start=(ci == 0), stop=(ci == ch_n - 1))
                for hg in range(2):
                    dst = out_sb[pr][:, (ss * 2 + hg) * 512:
                                     (ss * 2 + hg + 1) * 512]
                    if c0 == 0:
                        nc.vector.tensor_copy(dst, sc[hg][:])
                    else:
                        nc.vector.tensor_add(dst, dst, sc[hg][:])

    for pr in range(n_pairs):
        for ss in range(4):
            b = 2 * pr + (ss // 2)
            s0 = (ss % 2) * P
            eng = nc.sync if (pr * 4 + ss) % 2 == 0 else nc.scalar
            eng.dma_start(out_rows[b][s0:s0 + P, :],
                          out_sb[pr][:, ss * H:(ss + 1) * H])


def _build_kernel(n_pairs_tot, nreps=1):
    assert n_pairs_tot == sum(np_ for np_, _ in GROUPS)
    n_rows = 2 * n_pairs_tot
    nc = bacc.Bacc("TRN2", target_bir_lowering=False, debug=False,
                   num_devices=NCORES)
    xt_r = nc.dram_tensor("xt_r", [n_pairs_tot, P, HT * S2], MM_DT,
                          kind="ExternalInput").ap()
    wexp = []
    for gi, (np_, ni_) in enumerate(GROUPS):
        wi = ni_ * P
        wexp.append((
            nc.dram_tensor(f"wg_{gi}", [P, ni_ * HT * P], MM_DT,
                           kind="ExternalInput").ap(),
            nc.dram_tensor(f"wu_{gi}", [P, ni_ * HT * P], MM_DT,
                           kind="ExternalInput").ap(),
            nc.dram_tensor(f"wd_{gi}", [wi, H], MM_DT,
                           kind="ExternalInput").ap(),
        ))
    out_r = nc.dram_tensor("out_r", [n_rows, S, H], F32,
                           kind="ExternalOutput").ap()

    with tile.TileContext(nc) as tc, ExitStack() as ctx:
        pools = {
            "xt": ctx.enter_context(tc.tile_pool(name="xt", bufs=2)),
            "psgu": ctx.enter_context(
                tc.tile_pool(name="psgu", bufs=5, space="PSUM")),
            "pssc": ctx.enter_context(
                tc.tile_pool(name="pssc", bufs=3, space="PSUM")),
            "tmp": ctx.enter_context(tc.tile_pool(name="tmp", bufs=2)),
            "aT": ctx.enter_context(tc.tile_pool(name="aT", bufs=2)),
            "outsb": ctx.enter_context(tc.tile_pool(name="outsb", bufs=2)),
            "wg": ctx.enter_context(tc.tile_pool(name="wg", bufs=3)),
            "wu": ctx.enter_context(tc.tile_pool(name="wu", bufs=3)),
            "wd": ctx.enter_context(tc.tile_pool(name="wd", bufs=2 * CHUNK)),
        }

        for _rep in range(nreps):
            pair0 = 0
            for gi, (npair, ni_) in enumerate(GROUPS):
                prs = list(range(pair0, pair0 + npair))
                rows = list(range(2 * pair0, 2 * (pair0 + npair)))
                _expert_group(nc, pools,
                              [xt_r[p] for p in prs],
                              wexp[gi][0], wexp[gi][1], wexp[gi][2],
                              [out_r[r] for r in rows], npair, ni=ni_)
                pair0 += npair

    nc.compile()
    return nc


def build_nreps(R):
    """Rebuild the current-GROUPS kernel with the body repeated R times."""
    return _build_kernel(sum(np_ for np_, _ in GROUPS), nreps=R)


_KERNEL_CACHE = {}


def _get_kernel(groups):
    if groups not in _KERNEL_CACHE:
        global GROUPS
        GROUPS = groups
        _KERNEL_CACHE[groups] = _build_kernel(
            sum(np_ for np_, _ in groups))
    return _KERNEL_CACHE[groups]


def _routing(router_logits):
    """Replicate reference routing in numpy f32: softmax, top-2, renorm."""
    rl = np.asarray(router_logits, np.float32)
    m = rl.max(axis=-1, keepdims=True)
    ex = np.exp(rl - m, dtype=np.float32)
    rw = ex / ex.sum(axis=-1, keepdims=True)
    sel = np.argsort(-rw, axis=-1, kind="stable")[:, :TOP_K]
    w = np.take_along_axis(rw, sel, axis=-1)
    w = w / w.sum(axis=-1, keepdims=True)
    scale = np.float32(1.0 / NUM_MOE_LAYERS)
    w = scale * w + (np.float32(1.0) - scale) * w
    return sel, w.astype(np.float32)


def _ret_gu(w):
    """[H, NI*128] -> [P, NI*HT*128] i-tile-major lhsT layout (contig DMA)."""
    ni = w.shape[1] // P
    return np.ascontiguousarray(
        w.reshape(HT, P, ni, P).transpose(1, 2, 0, 3).reshape(P, -1))


def _ret_xpair(xa, xb):
    """Two [P, HT, S] tiles -> interleaved pair tile [P, HT*S2]."""
    return np.ascontiguousarray(
        np.concatenate([xa, xb], axis=2).reshape(P, HT * S2))


def kernel(x, router_logits, skill_gate, skill_up, skill_down,
           shared_gate, shared_up, shared_down):
    x = np.asarray(x, np.float32)
    skill_gate = np.asarray(skill_gate, np.float32).astype(NP_MM)
    skill_up = np.asarray(skill_up, np.float32).astype(NP_MM)
    skill_down = np.asarray(skill_down, np.float32).astype(NP_MM)
    shared_gate = np.asarray(shared_gate, np.float32).astype(NP_MM)
    shared_up = np.asarray(shared_up, np.float32).astype(NP_MM)
    shared_down = np.asarray(shared_down, np.float32).astype(NP_MM)

    sel, w = _routing(router_logits)
    lists = [[] for _ in range(E)]
    wmap = np.zeros((B, E), np.float32)
    for b in range(B):
        for k in range(TOP_K):
            e = int(sel[b, k])
            lists[e].append(b)
            wmap[b, e] = w[b, k]

    # decompose each expert's routed batches into weight-stream groups of
    # <=2 pairs; entries are (batch, is_real).  Two-pair groups are assigned
    # to one core each ("own" slots); leftover single pairs become
    # tensor-parallel slots split over I across ALL cores.
    groups2, groups1 = [], []
    for e in range(E):
        ent = [(b, True) for b in lists[e]]
        if len(ent) % 2:
            ent.append((0, False))
        pairs = [ent[i:i + 2] for i in range(0, len(ent), 2)]
        for i in range(0, len(pairs) - 1, 2):
            groups2.append((e, pairs[i] + pairs[i + 1]))
        if len(pairs) % 2:
            groups1.append((e, pairs[-1]))
    n2 = max(1, -(-len(groups2) // NCORES))
    n_tp = len(groups1)
    cfg = (((2, NIF),) * n2 + ((2, NIF),)          # own + shared slots
           + ((1, TPI // P),) * n_tp)              # tp slots
    dummy2 = (0, [(0, False)] * 4)
    groups2 += [dummy2] * (n2 * NCORES - len(groups2))

    # per-batch retiled xT [P, HT, S] bf16
    xt = x.transpose(0, 2, 1).astype(NP_MM)                # [B, H, S]
    xtt = np.ascontiguousarray(
        xt.reshape(B, HT, P, S).transpose(0, 2, 1, 3))     # [B, P, HT, S]

    # per-expert retiled gate/up (lhsT layout), shared likewise
    rt_g = [_ret_gu(skill_gate[e]) for e in range(E)]
    rt_u = [_ret_gu(skill_up[e]) for e in range(E)]
    rt_g_sh = _ret_gu(shared_gate)
    rt_u_sh = _ret_gu(shared_up)

    nc = _get_kernel(cfg)

    in_maps = []
    core_slots = []
    for c in range(NCORES):
        own = [groups2[c * n2 + j] for j in range(n2)]
        core_slots.append(own)
        pair_tiles = []
        for _, ent in own:
            for pi in range(0, len(ent), 2):
                pair_tiles.append(_ret_xpair(xtt[ent[pi][0]],
                                             xtt[ent[pi + 1][0]]))
        for j in range(0, 4, 2):                     # shared slot: 2 pairs
            pair_tiles.append(_ret_xpair(xtt[4 * c + j], xtt[4 * c + j + 1]))
        for _, ent in groups1:                       # tp slots
            pair_tiles.append(_ret_xpair(xtt[ent[0][0]], xtt[ent[1][0]]))
        m = {"xt_r": np.stack(pair_tiles)}
        for gi, (e, _) in enumerate(own):
            m[f"wg_{gi}"] = rt_g[e]
            m[f"wu_{gi}"] = rt_u[e]
            m[f"wd_{gi}"] = skill_down[e]
        m[f"wg_{n2}"] = rt_g_sh
        m[f"wu_{n2}"] = rt_u_sh
        m[f"wd_{n2}"] = shared_down
        ntp = TPI // P
        for tj, (e, _) in enumerate(groups1):
            gi = n2 + 1 + tj
            m[f"wg_{gi}"] = np.ascontiguousarray(
                rt_g[e].reshape(P, NIF, HT * P)
                [:, c * ntp:(c + 1) * ntp, :].reshape(P, -1))
            m[f"wu_{gi}"] = np.ascontiguousarray(
                rt_u[e].reshape(P, NIF, HT * P)
                [:, c * ntp:(c + 1) * ntp, :].reshape(P, -1))
            m[f"wd_{gi}"] = np.ascontiguousarray(
                skill_down[e][c * TPI:(c + 1) * TPI, :])
        in_maps.append(m)

    trace = bool(os.environ.get("TRNK_TRACE"))
    res = run_bass_kernel_spmd(nc, in_maps, core_ids=list(range(NCORES)),
                               trace=trace,
                               trace_cores=list(range(NCORES)) if trace else None)
    kernel.last_exec_time_ns = res.exec_time_ns
    kernel.last_results = res
    kernel.last_nc = nc
    kernel.last_in_maps = in_maps

    out = np.zeros((B, S, H), np.float32)
    for c in range(NCORES):
        r = res.results[c]["out_r"]
        row = 0
        for e, ent in core_slots[c]:
            for b, real in ent:
                if real:
                    out[b] += wmap[b, e] * r[row]
                row += 1
        for j in range(4):                            # shared slot rows
            out[4 * c + j] += r[row]
            row += 1
        n_own_rows = row
        # tp rows handled below (partial across cores)
    for tj, (e, ent) in enumerate(groups1):
        for k, (b, real) in enumerate(ent):
            if real:
                row = n_own_rows + 2 * tj + k
                part = sum(res.results[c]["out_r"][row]
                           for c in range(NCORES))
                out[b] += wmap[b, e] * part
    return out
